# revision 12
# baseline (speedup 1.0000x reference)
"""HQQ int4 weight-only quantized linear for TRN2, 8-core tensor-parallel.

out[M, N] = x[M, K] @ dequant(W_q[N, K]).T
  dequant: w[n, k] = (q[n, k] - 8) * scales[n, k//128] + zeros[n, k//128]

Sharding: column-parallel over N (out_features) across 8 NeuronCores;
x replicated; outputs concatenated on host. No collectives.

Weights are fully dequantized on the host.  The device kernel is a pure
streaming GEMM with a mixed-precision contraction:
  - first NGB k-groups in bf16: 26 k-tiles [128, n_shard] resident in SBUF
  - last NGF8 k-groups in fp8-e4m3 via DoubleRow perf mode (2 k-groups per
    matmul, ~2x PE throughput).  Scale split keeps PSUM exact: weights are
    quantized as e4m3(16*w), activations as e4m3(x/16), so partial products
    accumulate at the same scale as the bf16 groups.  Measured end-to-end
    rel-err ~1.6e-2 vs the 2e-2 gate.
  - x panels [128, *, 256] streamed (double-buffered)
  - psum[m128, n<=512] accumulated over all k-tiles, 6 banks in flight
  - junk-matmul warmup burst flips the PE HAM throttle to 8/8 while the
    first weight tiles stream in; startup DMAs are emitted in consumption
    order with explicit per-ring FIFO chains.
"""

import os
import sys

import numpy as np
import ml_dtypes

M = 4096
K = 4096
N = 11008
GROUP = 128
N_CORES = 8
N_SHARD = N // N_CORES  # 1376
NG = K // GROUP  # 32 k-groups of 128
NGF8 = 6  # trailing k-groups computed in fp8 DoubleRow (3 pair-matmuls)
NGB = NG - NGF8  # leading k-groups in bf16 (26)
NPAIR = NGF8 // 2
W8SCALE = 4.0  # w shipped as e4m3(4*w); x as e4m3(x/4)
M_PANEL = 256
BF16 = ml_dtypes.bfloat16
FP8 = ml_dtypes.float8_e4m3
N_WARM = 12  # junk matmuls (N=256) to warm the PE clock gate


def _install_axon_hooks_shim():
    """antenv.axon_hooks is missing from this image; run_bass_kernel_spmd
    imports it when tracing is requested (e.g. BASS_TRACE=1). Provide the
    same ctypes-based hook trn_boot would have registered."""
    import types

    try:
        import antenv.axon_hooks  # noqa: F401

        return
    except ImportError:
        pass
    try:
        import antenv
        from trn_agent_boot.trn_boot import _ntff_profile_via_ctypes

        hook = _ntff_profile_via_ctypes("/opt/axon/libaxon_pjrt.so")
        mod = types.ModuleType("antenv.axon_hooks")
        mod._hook = hook
        mod.get_axon_ntff_profile_hook = lambda: mod._hook

        def _set(h):
            mod._hook = h

        mod.set_axon_ntff_profile_hook = _set
        sys.modules["antenv.axon_hooks"] = mod
        antenv.axon_hooks = mod
    except Exception:
        pass


def build_bass(m=M, k=K, n_shard=N_SHARD, compile=True):
    import concourse.mybir as mybir
    import concourse.tile as tile
    from concourse.tile import add_dep_helper
    from concourse import bacc

    P = 128
    MP = M_PANEL
    assert m % MP == 0
    f32 = mybir.dt.float32
    bf16 = mybir.dt.bfloat16
    f8 = mybir.dt.float8e4
    DR = mybir.MatmulPerfMode.DoubleRow
    n_panels = m // MP
    nsub = MP // P  # m-subtiles per panel (2)

    nc = bacc.Bacc("TRN2", target_bir_lowering=False, debug=False)
    # x panels: bf16 part [128, NGB, 256]; fp8 part [128, NPAIR, 2, 256]
    xT4 = nc.dram_tensor("xT4", [n_panels, P, NGB, MP], bf16, kind="ExternalInput")
    x8T = nc.dram_tensor("x8T", [n_panels, P, NPAIR, 2, MP], f8, kind="ExternalInput")
    wd = nc.dram_tensor("wd", [NGB * P, n_shard], bf16, kind="ExternalInput")
    w8 = nc.dram_tensor("w8", [NPAIR, P, 2, n_shard], f8, kind="ExternalInput")
    out = nc.dram_tensor("out", [m, n_shard], bf16, kind="ExternalOutput")

    n_tiles = []
    st = 0
    while st < n_shard:
        nf = min(512, n_shard - st)
        n_tiles.append((st, nf))
        st += nf

    with tile.TileContext(nc) as tc:
        with (
            tc.tile_pool(name="wdeq", bufs=NGB // 2) as wdeq_pool,
            tc.tile_pool(name="w8p", bufs=NPAIR) as w8_pool,
            tc.tile_pool(name="warm", bufs=1) as warm_pool,
            tc.tile_pool(name="xp", bufs=3) as xp_pool,
            tc.tile_pool(name="x8p", bufs=3) as x8_pool,
            tc.tile_pool(name="osb", bufs=2) as osb_pool,
            tc.tile_pool(name="psum", bufs=6, space="PSUM") as psum_pool,
            tc.tile_pool(name="wps", bufs=1, space="PSUM") as wps_pool,
        ):
            # ---- PE warmup: junk matmuls while weight DMAs stream ----
            jnk = warm_pool.tile([P, 3 * P], bf16, tag="jnk")
            nc.vector.memset(jnk[:], 0.0)
            jps = wps_pool.tile([P, 256], f32, tag="jps")
            for _ in range(N_WARM):
                nc.tensor.matmul(
                    jps[:], jnk[:, :P], jnk[:, P : 3 * P], start=True, stop=True
                )

            # ---- startup DMAs in consumption order, alternated across the
            # two HWDGE rings; explicit ordering chains per ring so the Tile
            # scheduler cannot pull big x-panel transfers ahead of weight
            # k-tiles (they would steal SDMA round-robin bandwidth). ----
            xp_tiles = {}
            x8_tiles = {}
            xp_tiles[0] = xp_pool.tile([P, NGB, MP], bf16, tag="xp", name="xp0")
            x8_tiles[0] = x8_pool.tile([P, NPAIR, 2, MP], f8, tag="x8p", name="x8p0")
            wd_tiles = [None] * NGB
            w8_tiles = [None] * NPAIR
            # x-panel-0 chunk boundaries (k-groups), finer early
            xsplit = [(0, 4), (4, 12), (12, 20), (20, NGB)]
            # fp8 tiles first: each sweep consumes the fp8 pairs before the
            # bf16 groups, so the small fp8 tensors lead the supply stream
            seq = [("x8", None)] + [("w8", p) for p in range(NPAIR)]
            for lo, hi in xsplit:
                seq.append(("x0", (lo, hi)))
                for g2 in range(lo // 2, hi // 2):
                    seq.append(("wd", g2))
            last_on_ring = {0: None, 1: None}
            for pos, (kind, i) in enumerate(seq):
                r = pos % 2
                ring = nc.sync if r == 0 else nc.scalar
                if kind == "x0":
                    lo, hi = i
                    di = ring.dma_start(xp_tiles[0][:, lo:hi, :], xT4[0, :, lo:hi, :])
                elif kind == "wd":
                    wt = wdeq_pool.tile(
                        [P, 2, n_shard], bf16, tag="wdeq", name=f"wdp{i}"
                    )
                    di = ring.dma_start(
                        wt[:],
                        wd[i * 2 * P : (i + 1) * 2 * P, :].rearrange(
                            "(o p) n -> p o n", o=2
                        ),
                    )
                    wd_tiles[2 * i] = wt[:, 0, :]
                    wd_tiles[2 * i + 1] = wt[:, 1, :]
                elif kind == "x8":
                    di = ring.dma_start(x8_tiles[0][:], x8T[0])
                else:
                    wt = w8_pool.tile([P, 2, n_shard], f8, tag="w8p", name=f"w8_{i}")
                    di = ring.dma_start(wt[:], w8[i])
                    w8_tiles[i] = wt
                if last_on_ring[r] is not None:
                    add_dep_helper(
                        di.ins, last_on_ring[r].ins, sync=False, reason="dma order"
                    )
                last_on_ring[r] = di

            # second x panel: issue only after the weight stream
            xp_tiles[1] = xp_pool.tile([P, NGB, MP], bf16, tag="xp", name="xp1")
            x8_tiles[1] = x8_pool.tile([P, NPAIR, 2, MP], f8, tag="x8p", name="x8p1")
            d1b = nc.scalar.dma_start(x8_tiles[1][:], x8T[1])
            add_dep_helper(d1b.ins, last_on_ring[1].ins, sync=False, reason="x8p1 order")
            add_dep_helper(d1b.ins, last_on_ring[0].ins, sync=False, reason="x8p1 order")
            d1 = nc.scalar.dma_start(xp_tiles[1][:], xT4[1])
            add_dep_helper(d1.ins, d1b.ins, sync=False, reason="xp1 order")
            chain_after = [d1]  # third panel loads chain behind xp1

            # ---- matmul ----
            ecnt = [0]
            last_ms = m // P - 1

            def evict(psums, ms_abs, both_rings=False):
                osb = osb_pool.tile([P, n_shard], bf16, tag="osb")
                m0 = ms_abs * P
                for j, (st, nf) in enumerate(n_tiles):
                    if ecnt[0] % 2 == 0:
                        nc.vector.tensor_copy(osb[:, st : st + nf], psums[j])
                    else:
                        nc.scalar.copy(osb[:, st : st + nf], psums[j])
                    ring = nc.scalar if (both_rings and j % 2 == 1) else nc.sync
                    ecnt[0] += 1
                    ring.dma_start(
                        out[m0 : m0 + P, st : st + nf], osb[:, st : st + nf]
                    )

            def sweep_mms(psums, xp, x8, ms):
                for p in range(NPAIR):
                    lhsT = x8[:, p, :, ms * P : (ms + 1) * P]
                    for j, (st, nf) in enumerate(n_tiles):
                        nc.tensor.matmul(
                            psums[j],
                            lhsT,
                            w8_tiles[p][:, :, st : st + nf],
                            start=(p == 0),
                            stop=False,
                            perf_mode=DR,
                        )
                for g in range(NGB):
                    lhsT = xp[:, g, ms * P : (ms + 1) * P]
                    for j, (st, nf) in enumerate(n_tiles):
                        nc.tensor.matmul(
                            psums[j],
                            lhsT,
                            wd_tiles[g][:, st : st + nf],
                            start=False,
                            stop=(g == NGB - 1),
                        )

            def emit_panel_k_outer(xp, x8, mp):
                # both m-subtiles' k-sweeps interleaved: 6 open psum banks.
                pss = []
                for ms in range(nsub):
                    row = []
                    for j, (st, nf) in enumerate(n_tiles):
                        ps = psum_pool.tile([P, 512], f32, tag="ps", name="psA")[:, :nf]
                        row.append(ps)
                    pss.append(row)
                for p in range(NPAIR):
                    for ms in range(nsub):
                        lhsT = x8[:, p, :, ms * P : (ms + 1) * P]
                        for j, (st, nf) in enumerate(n_tiles):
                            nc.tensor.matmul(
                                pss[ms][j],
                                lhsT,
                                w8_tiles[p][:, :, st : st + nf],
                                start=(p == 0),
                                stop=False,
                                perf_mode=DR,
                            )
                for g in range(NGB):
                    for ms in range(nsub):
                        lhsT = xp[:, g, ms * P : (ms + 1) * P]
                        for j, (st, nf) in enumerate(n_tiles):
                            nc.tensor.matmul(
                                pss[ms][j],
                                lhsT,
                                wd_tiles[g][:, st : st + nf],
                                start=False,
                                stop=(g == NGB - 1),
                            )
                for ms in range(nsub):
                    evict(pss[ms], mp * nsub + ms)

            def emit_panel_ms_inner(xp, x8, mp):
                for ms in range(nsub):
                    psums = []
                    for j, (st, nf) in enumerate(n_tiles):
                        ps = psum_pool.tile([P, 512], f32, tag="ps", name="psB")[:, :nf]
                        psums.append(ps)
                    sweep_mms(psums, xp, x8, ms)
                    evict(psums, mp * nsub + ms, both_rings=(mp == n_panels - 1))

            for mp in range(n_panels):
                if mp not in xp_tiles:
                    x8_tiles[mp] = x8_pool.tile(
                        [P, NPAIR, 2, MP], f8, tag="x8p", name=f"x8p{mp}"
                    )
                    da = nc.scalar.dma_start(x8_tiles[mp][:], x8T[mp])
                    xp_tiles[mp] = xp_pool.tile(
                        [P, NGB, MP], bf16, tag="xp", name=f"xp{mp}"
                    )
                    db = nc.scalar.dma_start(xp_tiles[mp][:], xT4[mp])
                    if chain_after:
                        add_dep_helper(
                            da.ins, chain_after[-1].ins, sync=False, reason="xpfifo"
                        )
                        chain_after.clear()
                    add_dep_helper(db.ins, da.ins, sync=False, reason="xpfifo")
                if mp < 2:
                    emit_panel_k_outer(xp_tiles[mp], x8_tiles[mp], mp)
                else:
                    emit_panel_ms_inner(xp_tiles[mp], x8_tiles[mp], mp)

    if compile:
        nc.compile()
    return nc


def host_prep(x, W_q, scales, zeros, m=M, k=K):
    """Host-side layout prep + full dequantization of W.

    Returns xT4 (bf16 panels, leading NGB k-groups), x8T (fp8 panels,
    trailing NGF8 k-groups as DoubleRow pairs), wd_full (bf16 [NGB*128, N]),
    w8_full (fp8 [NPAIR, 128, 2, N])."""
    n = W_q.shape[0]
    nsh = n // N_CORES
    x = np.asarray(x)
    n_panels = m // M_PANEL
    kb = NGB * GROUP
    # x tiled: [panel, ki, g, m_in_panel]
    xt = x.reshape(n_panels, M_PANEL, NG, GROUP).transpose(0, 3, 2, 1)
    xT4 = np.ascontiguousarray(xt[:, :, :NGB, :])
    x8 = (xt[:, :, NGB:, :].astype(np.float32) / W8SCALE).astype(FP8)
    x8T = np.ascontiguousarray(x8.reshape(n_panels, GROUP, NPAIR, 2, M_PANEL))
    s = np.asarray(scales).astype(np.float32)
    z = np.asarray(zeros).astype(np.float32)
    w3 = np.asarray(W_q).reshape(n, NG, GROUP).astype(np.float32) - 8.0
    w3 = w3 * s[:, :, None] + z[:, :, None]  # [N, NG, G]
    wkn = w3.reshape(n, k).T  # [K, N] fp32
    wd_full = np.ascontiguousarray(wkn[:kb, :].astype(BF16))
    w8_full = np.ascontiguousarray(
        (wkn[kb:, :] * W8SCALE)
        .astype(FP8)
        .reshape(NPAIR, 2, GROUP, n)
        .transpose(0, 2, 1, 3)
    )  # [NPAIR, ki, 2, N]
    return xT4, x8T, wd_full, w8_full, nsh


_NC_CACHE = {}
_LAST_IN_MAPS = None


def kernel(x, W_q, scales, zeros):
    _install_axon_hooks_shim()
    from concourse.bass_utils import run_bass_kernel_spmd

    xT4, x8T, wd_full, w8_full, nsh = host_prep(x, W_q, scales, zeros)
    assert nsh == N_SHARD

    if "nc" not in _NC_CACHE:
        _NC_CACHE["nc"] = build_bass()
    nc = _NC_CACHE["nc"]

    in_maps = []
    for c in range(N_CORES):
        lo, hi = c * N_SHARD, (c + 1) * N_SHARD
        in_maps.append(
            {
                "xT4": xT4,
                "x8T": x8T,
                "wd": np.ascontiguousarray(wd_full[:, lo:hi]),
                "w8": np.ascontiguousarray(w8_full[:, :, :, lo:hi]),
            }
        )

    global _LAST_IN_MAPS
    _LAST_IN_MAPS = in_maps
    res = run_bass_kernel_spmd(nc, in_maps, list(range(N_CORES)))
    out = np.concatenate([res.results[c]["out"] for c in range(N_CORES)], axis=1)
    return out.astype(BF16, copy=False)
